# revision 40
# baseline (speedup 1.0000x reference)
"""Trainium2 Bass kernel for nn_CategoryInterestAttention.

Contract: kernel(**inputs) takes FULL unsharded inputs (as produced by the
problem's setup_inputs) and returns the FULL (512, 128) float32 output.

Strategy (pure data parallel, 8 NeuronCores, 64 batch rows each):
  - Each sequence token belongs to exactly one category group, so the
    attention is a segmented softmax: per token t only the score with its
    own group matters.  Per row r:
      qtokT = q_r^T @ M          (PE; M = group-match matrix, host-encoded)
      prod  = qtokT (.) kT       (DVE/Pool elementwise)
      s     = prod^T @ head_ind  (PE; per-head segment dot products)
      e     = exp(s * scale)     (Act; written into EV[:, 128:132])
      EV    = v_psum (.) e       (DVE/Pool; v read straight from PSUM)
      ctx|den = M_t^T @ EV       (PE; 2 matmuls per row, den cols for free)
    vs the dense formulation this removes the 64x redundant exp and the
    per-(head,chunk) 33-column context matmuls.
  - The k-projection bias (incl. folded LN bias) shifts all scores of a
    group by a constant -> dropped exactly (softmax shift invariance).
    The q bias needs a per-token correction; it is only emitted when the
    host-side folded bias is nonzero (it is zero for this problem).
  - Match matrices (two layouts), query gather indices, and present-group
    penalties are host-side re-encodings of sequence_cat_ids/mask ints.
  - LayerNorm gains/biases fold into projection weights host-side; bn_stats
    batched 4 tiles wide; elementwise work split across DVE and Pool.
  - bf16 matmuls with fp32 PSUM accumulation; final target-attention fp32.
"""

import numpy as np
import ml_dtypes

import concourse.bass as bass
import concourse.bacc as bacc
import concourse.tile as tile
from concourse import mybir
from concourse.bass_utils import run_bass_kernel_spmd

F32 = mybir.dt.float32
BF16 = mybir.dt.bfloat16
I32 = mybir.dt.int32
AF = mybir.ActivationFunctionType
OP = mybir.AluOpType

B, T, D = 512, 256, 128
C, H, L, F = 64, 4, 2, 512
HD = D // H                    # 32
NCORES = 8
R = B // NCORES                # 64 rows per core
NT = R * T                     # 16384 seq tokens per core
NX = R * C                     # 4096 group tokens per core
NTC = NT // 128                # 128 seq chunks
NXT = NX // 128                # 32 x-token tiles
SCALE_S = 1.0 / np.sqrt(np.float32(HD))
SCALE_L = 1.0 / np.sqrt(np.float32(D))
EPS = 1e-5


DEBUG = False


def _build(nc, has_bq):
    seq = nc.dram_tensor("seq", [NT, D], F32, kind="ExternalInput").ap()
    seqb = nc.dram_tensor("seqb", [NT, D], BF16, kind="ExternalInput").ap()
    mg = nc.dram_tensor("mg", [128, NXT * T], BF16, kind="ExternalInput").ap()
    mtg = nc.dram_tensor("mtg", [128, R * 2 * C], BF16,
                         kind="ExternalInput").ap()
    qidx = nc.dram_tensor("qidx", [128, NXT], I32, kind="ExternalInput").ap()
    pen = nc.dram_tensor("pen", [128, NXT], F32, kind="ExternalInput").ap()
    tgt = nc.dram_tensor("tgt", [R, D], F32, kind="ExternalInput").ap()
    hind = nc.dram_tensor("hind", [128, H], BF16, kind="ExternalInput").ap()
    ident = nc.dram_tensor("ident", [128, 128], F32, kind="ExternalInput").ap()
    identb = nc.dram_tensor("identb", [128, 128], BF16,
                            kind="ExternalInput").ap()
    io = dict(seq=seq, seqb=seqb, mg=mg, mtg=mtg, qidx=qidx, pen=pen, tgt=tgt,
              hind=hind, ident=ident, identb=identb)
    for name, shape, dt_ in [
        ("wkt", [D, D], BF16), ("wvt", [D, D], BF16), ("wqt", [D, D], BF16),
        ("wot", [D, D], BF16), ("w1t", [D, F], BF16), ("w2t", [D, F], BF16),
        ("bo", [D, 1], F32), ("b2", [D, 1], F32), ("b1_", [128, 4], F32),
    ] + ([("bqh", [D, H], BF16)] if has_bq else []):
        io[name] = [nc.dram_tensor(f"{name}{l}", shape, dt_,
                                   kind="ExternalInput").ap() for l in range(L)]
    io["out"] = nc.dram_tensor("out", [R, D], F32, kind="ExternalOutput").ap()
    if DEBUG:
        for nm, shape, dt_ in [
                ("dbg_x0", [128, NXT * D], F32), ("dbg_kT", [128, NT], BF16),
                ("dbg_q", [128, NXT * D], BF16),
                ("dbg_xl0", [128, NXT * D], F32), ("dbg_zT", [128, NT], BF16)]:
            io[nm] = nc.dram_tensor(nm, shape, dt_,
                                    kind="ExternalOutput").ap()

    with tile.TileContext(nc) as tc:
        from contextlib import ExitStack
        with ExitStack() as ctx:
            _body(ctx, tc, nc, io, has_bq)
    return nc


def _body(ctx, tc, nc, io, has_bq):
    P = 128
    persist = ctx.enter_context(tc.tile_pool(name="persist", bufs=1))
    consts = ctx.enter_context(tc.tile_pool(name="consts", bufs=1))
    ld = ctx.enter_context(tc.tile_pool(name="ld", bufs=3))
    small = ctx.enter_context(tc.tile_pool(name="small", bufs=4))
    ev = ctx.enter_context(tc.tile_pool(name="ev", bufs=4))
    ctokp = ctx.enter_context(tc.tile_pool(name="ctokp", bufs=5))
    epool = ctx.enter_context(tc.tile_pool(name="epool", bufs=4))
    prodp = ctx.enter_context(tc.tile_pool(name="prodp", bufs=4))
    psA = ctx.enter_context(tc.tile_pool(name="psA", bufs=2, space="PSUM"))
    psR = ctx.enter_context(tc.tile_pool(name="psR", bufs=3, space="PSUM"))
    psC = ctx.enter_context(tc.tile_pool(name="psC", bufs=2, space="PSUM"))
    psT = ctx.enter_context(tc.tile_pool(name="psT", bufs=1, space="PSUM"))

    # ---- tiles ----
    c_id = consts.tile([P, P], F32)
    c_idb = consts.tile([P, P], BF16)
    c_hind = consts.tile([P, H], BF16)
    c_eps = consts.tile([P, 1], F32)
    nc.vector.memset(c_eps, EPS)
    w = {}
    for name in ("wkt", "wvt", "wqt", "wot", "w1t", "w2t"):
        w[name] = [consts.tile(list(io[name][l].shape), BF16,
                               tag=f"{name}{l}", name=f"{name}{l}")
                   for l in range(L)]
    bias = {}
    for name in ("bo", "b2", "b1_") + (("bqh",) if has_bq else ()):
        bias[name] = [
            consts.tile(list(io[name][l].shape),
                        BF16 if name == "bqh" else F32,
                        tag=f"{name}{l}", name=f"{name}{l}")
            for l in range(L)]

    # ---- persistent data ----
    kT = persist.tile([P, NT], BF16)
    zT = persist.tile([P, NT], BF16)
    x_all = persist.tile([P, NXT, D], F32)
    xnT = persist.tile([P, NX], BF16)
    mgS = persist.tile([P, NXT, T], BF16)
    mtgS = persist.tile([P, R, 2, C], BF16)
    qidxS = persist.tile([P, NXT], I32)
    penS = persist.tile([P, NXT], F32)
    tbS = persist.tile([P, NXT, D], F32)
    q_sb = persist.tile([P, NXT, D], BF16)

    # input DMAs: qidx first (gates the gathers), gathers head the gpsimd
    # queue, bulky weights/constants follow behind them
    nc.sync.dma_start(out=qidxS, in_=io["qidx"])
    nc.sync.dma_start(out=c_idb, in_=io["identb"])
    nc.sync.dma_start(out=w["wkt"][0], in_=io["wkt"][0])
    for j in range(NXT):
        nc.gpsimd.indirect_dma_start(
            out=x_all[:, j, :], out_offset=None, in_=io["seq"][:],
            in_offset=bass.IndirectOffsetOnAxis(ap=qidxS[:, j:j + 1], axis=0))
    nc.scalar.dma_start(out=mgS, in_=io["mg"])
    nc.scalar.dma_start(out=mtgS, in_=io["mtg"])
    nc.gpsimd.dma_start(out=c_id, in_=io["ident"])
    nc.gpsimd.dma_start(out=c_hind, in_=io["hind"])
    for name in ("wkt", "wvt", "wqt", "wot", "w1t", "w2t"):
        for l in range(L):
            if name == "wkt" and l == 0:
                continue
            nc.gpsimd.dma_start(out=w[name][l], in_=io[name][l])
    for name in ("bo", "b2", "b1_") + (("bqh",) if has_bq else ()):
        for l in range(L):
            nc.gpsimd.dma_start(out=bias[name][l], in_=io[name][l])
    nc.gpsimd.dma_start(out=penS, in_=io["pen"])
    for half in range(2):
        nc.gpsimd.dma_start(
            out=tbS[64 * half:64 * half + 64, :, :],
            in_=bass.AP(tensor=io["tgt"].tensor, offset=half * D,
                        ap=[[0, 64], [2 * D, NXT], [1, D]]))

    if DEBUG:
        nc.sync.dma_start(out=io["dbg_x0"], in_=x_all)

    def vp(i):
        # SBUF-only ops may go to Pool; PSUM-touching ops must not.
        return nc.vector if i % 2 == 0 else nc.gpsimd

    def evict2(i):
        # PSUM -> SBUF evictions: only DVE and Act can read PSUM.
        return (nc.vector.tensor_copy, nc.scalar.copy)[i % 2]

    def pe_transpose_b(dst_sbuf_slice, src, ev_fn):
        pt = psT.tile([P, P], BF16, tag="pt")
        nc.tensor.transpose(out=pt, in_=src, identity=c_idb[:])
        ev_fn(out=dst_sbuf_slice, in_=pt)

    def norm_tile(i, out_ap, in_ap, mu, rsig, nm):
        """(x - mu) * rsig, alternating DVE tensor_scalar / Act identity."""
        if i % 3 == 0:
            nc.vector.tensor_scalar(out=out_ap, in0=in_ap, scalar1=mu,
                                    scalar2=rsig, op0=OP.subtract, op1=OP.mult)
        else:
            nc.scalar.activation(out=out_ap, in_=in_ap, func=AF.Identity,
                                 scale=rsig, bias=nm)

    # ---- stage 0: z = LN(seq) -> zT feature-major (bf16); kT for layer 0 ----
    # stats for 4 chunk-groups batched per sqrt to keep the Act table stable
    for g8 in range(NT // 2048):
        mvb = small.tile([P, 4, 4, 2], F32, tag="mvb")
        nm = small.tile([P, 4, 4, 1], F32, tag="nm")
        s4s = []
        for gg in range(4):
            g4 = 4 * g8 + gg
            s4 = ld.tile([P, 512], BF16, tag="seqld", bufs=6)
            nc.sync.dma_start(out=s4, in_=bass.AP(
                tensor=io["seqb"].tensor, offset=g4 * 512 * D,
                ap=[[512, 128], [1, 512]]))
            s4s.append(s4)
            for j in range(4):
                st = small.tile([P, 6], F32, tag="bnst")
                nc.vector.bn_stats(out=st, in_=s4[:, j * 128:(j + 1) * 128])
                nc.vector.bn_aggr(out=mvb[:, gg, j, :], in_=st)
        nc.scalar.activation(out=mvb[:, :, :, 1:2], in_=mvb[:, :, :, 1:2],
                             func=AF.Sqrt, bias=c_eps[:])
        nc.vector.reciprocal(out=mvb[:, :, :, 1:2], in_=mvb[:, :, :, 1:2])
        nc.vector.scalar_tensor_tensor(
            out=nm, in0=mvb[:, :, :, 0:1], scalar=-1.0,
            in1=mvb[:, :, :, 1:2], op0=OP.mult, op1=OP.mult)
        for gg in range(4):
            g4 = 4 * g8 + gg
            s4 = s4s[gg]
            z4 = ld.tile([P, 512], BF16, tag="ztok", bufs=3)
            pt4 = psT.tile([P, 512], BF16, tag="pt")
            for j in range(4):
                norm_tile(g4 * 4 + j, z4[:, j * 128:(j + 1) * 128],
                          s4[:, j * 128:(j + 1) * 128], mvb[:, gg, j, 0:1],
                          mvb[:, gg, j, 1:2], nm[:, gg, j, :])
                nc.tensor.transpose(out=pt4[:, j * 128:(j + 1) * 128],
                                    in_=z4[:, j * 128:(j + 1) * 128],
                                    identity=c_idb[:])
            evict2(g4)(out=zT[:, g4 * 512:(g4 + 1) * 512], in_=pt4)
            ps = psA.tile([P, 512], F32, tag="mm")
            nc.tensor.matmul(out=ps, lhsT=w["wkt"][0][:],
                             rhs=zT[:, g4 * 512:(g4 + 1) * 512],
                             start=True, stop=True)
            evict2(g4 + 1)(out=kT[:, g4 * 512:(g4 + 1) * 512], in_=ps)

    if DEBUG:
        nc.sync.dma_start(out=io["dbg_zT"], in_=zT)
        nc.sync.dma_start(out=io["dbg_kT"], in_=kT)

    # ---- x layernorm -> xnT (feature-major bf16) ----
    def ln_to(dst_T):
        mvb = small.tile([P, NXT, 2], F32, tag="mvb2")
        nm = small.tile([P, NXT, 1], F32, tag="nm2")
        for jj in range(NXT):
            st = small.tile([P, 6], F32, tag="bnst2")
            nc.vector.bn_stats(out=st, in_=x_all[:, jj, :])
            nc.vector.bn_aggr(out=mvb[:, jj, :], in_=st)
        nc.scalar.activation(out=mvb[:, :, 1:2], in_=mvb[:, :, 1:2],
                             func=AF.Sqrt, bias=c_eps[:])
        nc.vector.reciprocal(out=mvb[:, :, 1:2], in_=mvb[:, :, 1:2])
        nc.vector.scalar_tensor_tensor(
            out=nm, in0=mvb[:, :, 0:1], scalar=-1.0,
            in1=mvb[:, :, 1:2], op0=OP.mult, op1=OP.mult)
        for g4 in range(NXT // 4):
            z4 = ld.tile([P, 512], BF16, tag="zx")
            pt4 = psT.tile([P, 512], BF16, tag="pt")
            for j in range(4):
                jj = 4 * g4 + j
                norm_tile(jj, z4[:, j * 128:(j + 1) * 128], x_all[:, jj, :],
                          mvb[:, jj, 0:1], mvb[:, jj, 1:2], nm[:, jj, :])
                nc.tensor.transpose(out=pt4[:, j * 128:(j + 1) * 128],
                                    in_=z4[:, j * 128:(j + 1) * 128],
                                    identity=c_idb[:])
            evict2(g4)(out=dst_T[:, g4 * 512:(g4 + 1) * 512], in_=pt4)

    for l in range(L):
        # ---- kT = Wk' @ z (feature-major); layer 0 done in stage 0 ----
        for nn in (() if l == 0 else range(NT // 512)):
            ps = psA.tile([P, 512], F32, tag="mm")
            nc.tensor.matmul(out=ps, lhsT=w["wkt"][l][:],
                             rhs=zT[:, nn * 512:(nn + 1) * 512],
                             start=True, stop=True)
            nc.scalar.copy(out=kT[:, nn * 512:(nn + 1) * 512], in_=ps)
        # ---- x LN + q projection (token-major, 2 rows x 64 groups/tile) ----
        ln_to(xnT)
        for j4 in range(NXT // 4):
            psq = psA.tile([P, 512], F32, tag="mm")
            for j in range(4):
                nc.tensor.matmul(
                    out=psq[:, j * 128:(j + 1) * 128],
                    lhsT=xnT[:, (4 * j4 + j) * 128:(4 * j4 + j + 1) * 128],
                    rhs=w["wqt"][l][:], start=True, stop=True)
            evict2(j4)(out=q_sb[:, 4 * j4:4 * j4 + 4, :], in_=psq)
        if DEBUG and l == 0:
            nc.sync.dma_start(out=io["dbg_q"], in_=q_sb)
        # ---- attention, one row pair at a time ----
        cts = []
        for rp in range(NXT):
            pc = psC.tile([P, 148], F32, tag="ctx")
            sps = pc[:, 132:148]
            qvs = []
            for half in range(2):
                r = 2 * rp + half
                qv = psR.tile([P, 512], F32, tag="qv")
                qvs.append(qv)
                nc.tensor.matmul(
                    out=qv[:, 0:256],
                    lhsT=q_sb[64 * half:64 * half + 64, rp, :],
                    rhs=mgS[64 * half:64 * half + 64, rp, :],
                    start=True, stop=True)
                for c in range(2):
                    cc = 2 * r + c
                    nc.tensor.matmul(
                        out=qv[:, 256 + c * 128:256 + (c + 1) * 128],
                        lhsT=zT[:, cc * 128:(cc + 1) * 128],
                        rhs=w["wvt"][l][:], start=True, stop=True)
                prod = prodp.tile([P, 2, D], BF16, tag="prod")
                nc.vector.tensor_tensor(
                    out=prod[:].rearrange("p c d -> p (c d)"),
                    in0=qv[:, 0:256],
                    in1=kT[:, 2 * r * 128:(2 * r + 2) * 128], op=OP.mult)
                for c in range(2):
                    k = 2 * half + c
                    nc.tensor.matmul(out=sps[:, k * 4:(k + 1) * 4],
                                     lhsT=prod[:, c, :], rhs=c_hind[:],
                                     start=True, stop=not has_bq)
                    if has_bq:
                        nc.tensor.matmul(
                            out=sps[:, k * 4:(k + 1) * 4],
                            lhsT=kT[:, (2 * r + c) * 128:(2 * r + c + 1) * 128],
                            rhs=bias["bqh"][l][:], start=False, stop=True)
            EV = epool.tile([P, 4, H * HD + H], BF16, tag="EV")
            nc.scalar.activation(
                out=EV[:, :, 128:132],
                in_=sps[:].rearrange("p (k h) -> p k h", k=4),
                func=AF.Exp, scale=float(SCALE_S))
            for half in range(2):
                r = 2 * rp + half
                nc.vector.tensor_tensor(
                    out=EV[:, 2 * half:2 * half + 2, 0:128].rearrange(
                        "p c (h d) -> p c h d", h=H),
                    in0=qvs[half][:, 256:512].rearrange(
                        "p (c h d) -> p c h d", c=2, h=H),
                    in1=EV[:, 2 * half:2 * half + 2, 128:132].rearrange(
                        "p c (h o) -> p c h o", o=1).to_broadcast(
                            [P, 2, H, HD]),
                    op=OP.mult)
                for c in range(2):
                    nc.tensor.matmul(
                        out=pc[64 * half:64 * half + 64, 0:132],
                        lhsT=mtgS[:, r, c, :], rhs=EV[:, 2 * half + c, :],
                        start=(c == 0), stop=(c == 1))
            rd = small.tile([P, H, 1], F32, tag="rd")
            nc.vector.tensor_scalar(
                out=rd, in0=pc[:, 128:132].rearrange("p (h o) -> p h o", o=1),
                scalar1=1e-30, scalar2=None, op0=OP.add)
            nc.vector.reciprocal(out=rd, in_=rd)
            ct = ctokp.tile([P, D], BF16, tag="ctok")
            nc.vector.scalar_tensor_tensor(
                out=ct[:].rearrange("p (h d) -> p h d", h=H),
                in0=pc[:, 0:128].rearrange("p (h d) -> p h d", h=H),
                scalar=1.0, in1=rd[:].to_broadcast([P, H, HD]),
                op0=OP.mult, op1=OP.mult)
            cts.append(ct)
            if rp % 4 == 3:
                sl = rp // 4
                cT = ev.tile([P, 512], BF16, tag="cT")
                ptc = psT.tile([P, 512], BF16, tag="pt")
                for k in range(4):
                    nc.tensor.transpose(out=ptc[:, k * 128:(k + 1) * 128],
                                        in_=cts[k][:], identity=c_idb[:])
                evict2(sl)(out=cT, in_=ptc)
                cts = []
                ps = psA.tile([P, 512], F32, tag="mm")
                nc.tensor.matmul(out=ps, lhsT=w["wot"][l][:], rhs=cT,
                                 start=True, stop=True)
                aoT = ev.tile([P, 512], BF16, tag="aoT")
                nc.scalar.activation(out=aoT, in_=ps, func=AF.Identity,
                                     bias=bias["bo"][l][:])
                pt4 = psT.tile([P, 512], BF16, tag="pt")
                for k in range(4):
                    nc.tensor.transpose(out=pt4[:, k * 128:(k + 1) * 128],
                                        in_=aoT[:, k * 128:(k + 1) * 128],
                                        identity=c_idb[:])
                nc.vector.tensor_tensor(
                    out=x_all[:, 4 * sl:4 * sl + 4, :],
                    in0=x_all[:, 4 * sl:4 * sl + 4, :],
                    in1=pt4[:].rearrange("p (j d) -> p j d", j=4), op=OP.add)

        # ---- FFN ----
        ln_to(xnT)
        for nn in range(NX // 512):
            r1 = []
            for fc in range(4):
                ps = psA.tile([P, 512], F32, tag="mm")
                nc.tensor.matmul(out=ps,
                                 lhsT=w["w1t"][l][:, fc * 128:(fc + 1) * 128],
                                 rhs=xnT[:, nn * 512:(nn + 1) * 512],
                                 start=True, stop=True)
                r1t = ev.tile([P, 512], BF16, tag="r1")
                if fc % 2 == 0:
                    nc.scalar.activation(out=r1t, in_=ps, func=AF.Relu,
                                         bias=bias["b1_"][l][:, fc:fc + 1])
                else:
                    nc.vector.tensor_scalar(out=r1t, in0=ps,
                                            scalar1=bias["b1_"][l][:, fc:fc + 1],
                                            scalar2=0.0, op0=OP.add, op1=OP.max)
                r1.append(r1t)
            ps2 = psA.tile([P, 512], F32, tag="mm")
            for fc in range(4):
                nc.tensor.matmul(out=ps2,
                                 lhsT=w["w2t"][l][:, fc * 128:(fc + 1) * 128],
                                 rhs=r1[fc], start=(fc == 0), stop=(fc == 3))
            f2T = ev.tile([P, 512], BF16, tag="aoT")
            nc.scalar.activation(out=f2T, in_=ps2, func=AF.Identity,
                                 bias=bias["b2"][l][:])
            pt4 = psT.tile([P, 512], BF16, tag="pt")
            for k in range(4):
                nc.tensor.transpose(out=pt4[:, k * 128:(k + 1) * 128],
                                    in_=f2T[:, k * 128:(k + 1) * 128],
                                    identity=c_idb[:])
            nc.vector.tensor_tensor(
                out=x_all[:, 4 * nn:4 * nn + 4, :],
                in0=x_all[:, 4 * nn:4 * nn + 4, :],
                in1=pt4[:].rearrange("p (j d) -> p j d", j=4), op=OP.add)
        if DEBUG and l == 0:
            nc.sync.dma_start(out=io["dbg_xl0"], in_=x_all)

    # ---- final stage (fp32): logits, softmax over groups, weighted sum ----
    Lpair = persist.tile([P, NXT], F32)
    for gg in range(4):
        sc = ld.tile([P, 8, D], F32, tag="fsc")
        vp(gg).tensor_tensor(out=sc, in0=x_all[:, 8 * gg:8 * gg + 8, :],
                             in1=tbS[:, 8 * gg:8 * gg + 8, :], op=OP.mult)
        nc.vector.tensor_reduce(
            out=Lpair[:, 8 * gg:8 * gg + 8].rearrange("p (j o) -> p j o", o=1),
            in_=sc, axis=mybir.AxisListType.X, op=OP.add)
    Lgr = persist.tile([P, R], F32)
    nc.vector.memset(Lgr, -1e9)
    for par in range(2):
        lg = Lgr[64 * par:64 * par + 64, :].rearrange("p (j two) -> p j two",
                                                      two=2)
        nc.vector.scalar_tensor_tensor(
            out=lg[:, :, par:par + 1],
            in0=Lpair[64 * par:64 * par + 64, :].rearrange(
                "p (j o) -> p j o", o=1),
            scalar=float(SCALE_L),
            in1=penS[64 * par:64 * par + 64, :].rearrange(
                "p (j o) -> p j o", o=1),
            op0=OP.mult, op1=OP.add)
    psL = psC.tile([R, P], F32, tag="ctx")
    nc.tensor.transpose(out=psL, in_=Lgr, identity=c_id[:])
    Erg = persist.tile([R, P], F32)
    den = small.tile([R, 1], F32, tag="den")
    nc.scalar.activation(out=Erg, in_=psL, func=AF.Exp, accum_out=den)
    nc.vector.reciprocal(out=den, in_=den)
    nc.vector.tensor_scalar(out=Erg, in0=Erg, scalar1=den, scalar2=None,
                            op0=OP.mult)
    psW = psC.tile([P, R], F32, tag="ctx")
    nc.tensor.transpose(out=psW, in_=Erg, identity=c_id[0:R, 0:R])
    wT = persist.tile([P, R], F32)
    nc.vector.tensor_copy(out=wT, in_=psW)
    for a in range(NXT // 4):
        psO = psC.tile([2, 512], F32, tag="ctx")
        for k in range(4):
            j = 4 * a + k
            nc.tensor.matmul(out=psO[:, k * 128:(k + 1) * 128],
                             lhsT=wT[:, 2 * j:2 * j + 2],
                             rhs=x_all[:, j, :], start=True, stop=True)
        o4 = ev.tile([2, 512], F32, tag="osb")
        evict2(a)(out=o4, in_=psO)
        eng = (nc.sync, nc.scalar, nc.gpsimd)[a % 3]
        eng.dma_start(
            out=bass.AP(tensor=io["out"].tensor, offset=8 * a * D,
                        ap=[[D, 2], [2 * D, 4], [1, D]]),
            in_=o4)


# ---------------------------------------------------------------------------
# host side
# ---------------------------------------------------------------------------

_NC_CACHE = {}


def _get_nc(has_bq=False):
    key = ("nc", has_bq)
    if key not in _NC_CACHE:
        nc = bacc.Bacc("TRN2", target_bir_lowering=False, debug=False,
                       enable_asserts=False)
        _build(nc, has_bq)
        nc.compile()
        _NC_CACHE[key] = nc
    return _NC_CACHE[key]


def _consts():
    ident = np.eye(128, dtype=np.float32)
    identb = np.eye(128, dtype=ml_dtypes.bfloat16)
    hind = np.zeros((128, H), np.float32)
    for h in range(H):
        hind[h * HD:(h + 1) * HD, h] = 1.0
    return dict(ident=ident, identb=identb,
                hind=np.ascontiguousarray(hind.astype(ml_dtypes.bfloat16)))


def _prep_weights(inp):
    wqkv = np.asarray(inp["wqkv"], np.float32)
    bqkv = np.asarray(inp["bqkv"], np.float32)
    wo = np.asarray(inp["wo"], np.float32)
    bo = np.asarray(inp["bo"], np.float32)
    l1g = np.asarray(inp["ln1_g"], np.float32)
    l1b = np.asarray(inp["ln1_b"], np.float32)
    l2g = np.asarray(inp["ln2_g"], np.float32)
    l2b = np.asarray(inp["ln2_b"], np.float32)
    w1 = np.asarray(inp["w1"], np.float32)
    b1 = np.asarray(inp["b1"], np.float32)
    w2 = np.asarray(inp["w2"], np.float32)
    b2 = np.asarray(inp["b2"], np.float32)
    Wq, Wk, Wv = wqkv[:, :D], wqkv[:, D:2 * D], wqkv[:, 2 * D:]
    bq_, bk_, bv_ = bqkv[:, :D], bqkv[:, D:2 * D], bqkv[:, 2 * D:]
    bf = lambda x: np.ascontiguousarray(x.astype(ml_dtypes.bfloat16))
    f32 = lambda x: np.ascontiguousarray(x.astype(np.float32))
    m = {}
    has_bq = False
    for l in range(L):
        Wqp = Wq[l] * l1g[l][None, :]
        Wkp = Wk[l] * l1g[l][None, :]
        Wvp = Wv[l] * l1g[l][None, :]
        W1p = w1[l] * l2g[l][None, :]
        bqp = Wq[l] @ l1b[l] + bq_[l]
        bvp = Wv[l] @ l1b[l] + bv_[l]
        b1p = w1[l] @ l2b[l] + b1[l]
        bop = wo[l] @ bvp + bo[l]          # v bias folded through wo
        # k bias dropped exactly: constant per (group, head) under softmax
        m[f"wkt{l}"] = bf(Wkp.T)
        m[f"wvt{l}"] = bf(Wvp.T)
        m[f"wqt{l}"] = bf(Wqp.T)
        m[f"wot{l}"] = bf(wo[l].T)
        m[f"w1t{l}"] = bf(W1p.T)
        w2tl = np.empty((128, F), np.float32)
        for fc in range(4):
            w2tl[:, fc * 128:(fc + 1) * 128] = w2[l][:, fc * 128:(fc + 1) * 128].T
        m[f"w2t{l}"] = bf(w2tl)
        m[f"bo{l}"] = f32(bop[:, None])
        m[f"b2{l}"] = f32(b2[l][:, None])
        m[f"b1_{l}"] = f32(b1p.reshape(4, 128).T)
        if np.any(bqp != 0.0):
            has_bq = True
        bqh = np.zeros((D, H), np.float32)
        for h in range(H):
            bqh[h * HD:(h + 1) * HD, h] = bqp[h * HD:(h + 1) * HD]
        m[f"bqh{l}"] = bf(bqh)
    if not has_bq:
        for l in range(L):
            del m[f"bqh{l}"]
    return m, has_bq


def _prep_row_data(catm):
    """Per-core encodings of the category/mask ints.

    catm: (R, T) int32 with -1 for masked positions.
    Returns mg (128, NXT*T) bf16, mtg (128, R*2*C) bf16, qidx (128, NXT) i32,
    pen (128, NXT) f32.
    """
    g = np.arange(C)
    match = (catm[:, None, :] == g[None, :, None])          # (R, C, T) bool
    mb = match.astype(ml_dtypes.bfloat16)
    mg = np.ascontiguousarray(
        mb.reshape(NXT, 2, C, T).transpose(1, 2, 0, 3)).reshape(128, NXT * T)
    mtg = np.ascontiguousarray(
        mb.reshape(R, C, 2, 128).transpose(3, 0, 2, 1)).reshape(128, R * 2 * C)
    pos = (np.arange(T, dtype=np.int64) + 1) * match        # (R, C, T)
    qpos = pos.max(-1)                                      # (R, C)
    qi = (np.clip(qpos - 1, 0, T - 1) +
          T * np.arange(R, dtype=np.int64)[:, None]).astype(np.int32)
    qidx = np.ascontiguousarray(
        qi.reshape(NXT, 2, C).transpose(1, 2, 0)).reshape(128, NXT)
    present = match.any(-1).astype(np.float32)              # (R, C)
    penv = (present - 1.0) * 1e9
    pen = np.ascontiguousarray(
        penv.reshape(NXT, 2, C).transpose(1, 2, 0)).reshape(128, NXT)
    return mg, mtg, qidx, pen


def kernel(**inputs):
    wm, has_bq = _prep_weights(inputs)
    nc = _get_nc(has_bq)
    cm = _consts()
    seq = np.asarray(inputs["sequence_item_emb"], np.float32)
    cat = np.asarray(inputs["sequence_cat_ids"])
    msk = np.asarray(inputs["sequence_mask"])
    tgt = np.asarray(inputs["target_item_emb"], np.float32)
    in_maps = []
    for i in range(NCORES):
        rs = slice(i * R, (i + 1) * R)
        im = dict(wm)
        im.update(cm)
        im["seq"] = np.ascontiguousarray(seq[rs].reshape(NT, D))
        im["seqb"] = np.ascontiguousarray(
            im["seq"].astype(ml_dtypes.bfloat16).reshape(32, 4, 128, D)
            .transpose(0, 2, 1, 3)).reshape(NT, D)
        catm = np.where(msk[rs], cat[rs], -1).astype(np.int32)
        mg, mtg, qidx, pen = _prep_row_data(catm)
        im["mg"], im["mtg"], im["qidx"], im["pen"] = mg, mtg, qidx, pen
        im["tgt"] = np.ascontiguousarray(tgt[rs])
        in_maps.append(im)
    res = run_bass_kernel_spmd(nc, in_maps, list(range(NCORES)))
    _NC_CACHE["last"] = res
    return np.concatenate([res.results[i]["out"] for i in range(NCORES)], axis=0)


# revision 43
# speedup vs baseline: 1.1204x; 1.1204x over previous
"""Trainium2 Bass kernel for nn_CategoryInterestAttention.

Contract: kernel(**inputs) takes FULL unsharded inputs (as produced by the
problem's setup_inputs) and returns the FULL (512, 128) float32 output.

Strategy (pure data parallel, 8 NeuronCores, 64 batch rows each):
  - Each sequence token belongs to exactly one category group, so the
    attention is a segmented softmax: per token t only the score with its
    own group matters.  Per row r:
      qtokT = q_r^T @ M          (PE; M = group-match matrix, host-encoded)
      prod  = qtokT (.) kT       (DVE/Pool elementwise)
      s     = prod^T @ head_ind  (PE; per-head segment dot products)
      e     = exp(s * scale)     (Act; written into EV[:, 128:132])
      EV    = v_psum (.) e       (DVE/Pool; v read straight from PSUM)
      ctx|den = M_t^T @ EV       (PE; 2 matmuls per row, den cols for free)
    vs the dense formulation this removes the 64x redundant exp and the
    per-(head,chunk) 33-column context matmuls.
  - The k-projection bias (incl. folded LN bias) shifts all scores of a
    group by a constant -> dropped exactly (softmax shift invariance).
    The q bias needs a per-token correction; it is only emitted when the
    host-side folded bias is nonzero (it is zero for this problem).
  - Match matrices (two layouts), query gather indices, and present-group
    penalties are host-side re-encodings of sequence_cat_ids/mask ints.
  - LayerNorm gains/biases fold into projection weights host-side; bn_stats
    batched 4 tiles wide; elementwise work split across DVE and Pool.
  - bf16 matmuls with fp32 PSUM accumulation; final target-attention fp32.
"""

import numpy as np
import ml_dtypes

import concourse.bass as bass
import concourse.bacc as bacc
import concourse.tile as tile
from concourse import mybir
from concourse.bass_utils import run_bass_kernel_spmd

F32 = mybir.dt.float32
BF16 = mybir.dt.bfloat16
I32 = mybir.dt.int32
AF = mybir.ActivationFunctionType
OP = mybir.AluOpType

B, T, D = 512, 256, 128
C, H, L, F = 64, 4, 2, 512
HD = D // H                    # 32
NCORES = 8
R = B // NCORES                # 64 rows per core
NT = R * T                     # 16384 seq tokens per core
NX = R * C                     # 4096 group tokens per core
NTC = NT // 128                # 128 seq chunks
NXT = NX // 128                # 32 x-token tiles
SCALE_S = 1.0 / np.sqrt(np.float32(HD))
SCALE_L = 1.0 / np.sqrt(np.float32(D))
EPS = 1e-5


DEBUG = False


def _build(nc, has_bq):
    seq = nc.dram_tensor("seq", [NT, D], F32, kind="ExternalInput").ap()
    seqb = nc.dram_tensor("seqb", [NT, D], BF16, kind="ExternalInput").ap()
    mg = nc.dram_tensor("mg", [128, NXT * T], BF16, kind="ExternalInput").ap()
    mtg = nc.dram_tensor("mtg", [128, R * 2 * C], BF16,
                         kind="ExternalInput").ap()
    qidx = nc.dram_tensor("qidx", [128, NXT], I32, kind="ExternalInput").ap()
    pen = nc.dram_tensor("pen", [128, NXT], F32, kind="ExternalInput").ap()
    tgt = nc.dram_tensor("tgt", [R, D], F32, kind="ExternalInput").ap()
    hind = nc.dram_tensor("hind", [128, H], BF16, kind="ExternalInput").ap()
    ident = nc.dram_tensor("ident", [128, 128], F32, kind="ExternalInput").ap()
    identb = nc.dram_tensor("identb", [128, 128], BF16,
                            kind="ExternalInput").ap()
    io = dict(seq=seq, seqb=seqb, mg=mg, mtg=mtg, qidx=qidx, pen=pen, tgt=tgt,
              hind=hind, ident=ident, identb=identb)
    for name, shape, dt_ in [
        ("wkt", [D, D], BF16), ("wvt", [D, D], BF16), ("wqt", [D, D], BF16),
        ("wot", [D, D], BF16), ("w1t", [D, F], BF16), ("w2t", [D, F], BF16),
        ("bo", [D, 1], F32), ("b2", [D, 1], F32), ("b1_", [128, 4], F32),
    ] + ([("bqh", [D, H], BF16)] if has_bq else []):
        io[name] = [nc.dram_tensor(f"{name}{l}", shape, dt_,
                                   kind="ExternalInput").ap() for l in range(L)]
    io["out"] = nc.dram_tensor("out", [R, D], F32, kind="ExternalOutput").ap()
    if DEBUG:
        for nm, shape, dt_ in [
                ("dbg_x0", [128, NXT * D], F32), ("dbg_kT", [128, NT], BF16),
                ("dbg_q", [128, NXT * D], BF16),
                ("dbg_xl0", [128, NXT * D], F32), ("dbg_zT", [128, NT], BF16)]:
            io[nm] = nc.dram_tensor(nm, shape, dt_,
                                    kind="ExternalOutput").ap()

    with tile.TileContext(nc) as tc:
        from contextlib import ExitStack
        with ExitStack() as ctx:
            _body(ctx, tc, nc, io, has_bq)
    return nc


def _body(ctx, tc, nc, io, has_bq):
    P = 128
    persist = ctx.enter_context(tc.tile_pool(name="persist", bufs=1))
    consts = ctx.enter_context(tc.tile_pool(name="consts", bufs=1))
    ld = ctx.enter_context(tc.tile_pool(name="ld", bufs=3))
    small = ctx.enter_context(tc.tile_pool(name="small", bufs=2))
    ev = ctx.enter_context(tc.tile_pool(name="ev", bufs=4))
    ctokp = ctx.enter_context(tc.tile_pool(name="ctokp", bufs=5))
    epool = ctx.enter_context(tc.tile_pool(name="epool", bufs=4))
    prodp = ctx.enter_context(tc.tile_pool(name="prodp", bufs=4))
    psA = ctx.enter_context(tc.tile_pool(name="psA", bufs=2, space="PSUM"))
    psR = ctx.enter_context(tc.tile_pool(name="psR", bufs=3, space="PSUM"))
    psC = ctx.enter_context(tc.tile_pool(name="psC", bufs=2, space="PSUM"))
    psT = ctx.enter_context(tc.tile_pool(name="psT", bufs=1, space="PSUM"))

    # ---- tiles ----
    c_id = consts.tile([P, P], F32)
    c_idb = consts.tile([P, P], BF16)
    c_hind = consts.tile([P, H], BF16)
    c_eps = consts.tile([P, 1], F32)
    nc.vector.memset(c_eps, EPS)
    w = {}
    for name in ("wkt", "wvt", "wqt", "wot", "w1t", "w2t"):
        w[name] = [consts.tile(list(io[name][l].shape), BF16,
                               tag=f"{name}{l}", name=f"{name}{l}")
                   for l in range(L)]
    bias = {}
    for name in ("bo", "b2", "b1_") + (("bqh",) if has_bq else ()):
        bias[name] = [
            consts.tile(list(io[name][l].shape),
                        BF16 if name == "bqh" else F32,
                        tag=f"{name}{l}", name=f"{name}{l}")
            for l in range(L)]

    # ---- persistent data ----
    kT = persist.tile([P, NT], BF16)
    zT = persist.tile([P, NT], BF16)
    x_all = persist.tile([P, NXT, D], F32)
    xnT = persist.tile([P, NX], BF16)
    mgS = persist.tile([P, NXT, T], BF16)
    mtgS = persist.tile([P, R, 2, C], BF16)
    qidxS = persist.tile([P, NXT], I32)
    penS = persist.tile([P, NXT], F32)
    tbS = persist.tile([P, NXT, D], F32)
    q_sb = persist.tile([P, NXT, D], BF16)

    # input DMAs: qidx first (gates the gathers), gathers head the gpsimd
    # queue, bulky weights/constants follow behind them
    nc.sync.dma_start(out=qidxS, in_=io["qidx"])
    nc.sync.dma_start(out=c_idb, in_=io["identb"])
    nc.sync.dma_start(out=w["wkt"][0], in_=io["wkt"][0])
    for j in range(NXT):
        nc.gpsimd.indirect_dma_start(
            out=x_all[:, j, :], out_offset=None, in_=io["seq"][:],
            in_offset=bass.IndirectOffsetOnAxis(ap=qidxS[:, j:j + 1], axis=0))
    nc.scalar.dma_start(out=mgS, in_=io["mg"])
    nc.scalar.dma_start(out=mtgS, in_=io["mtg"])
    nc.gpsimd.dma_start(out=c_id, in_=io["ident"])
    nc.gpsimd.dma_start(out=c_hind, in_=io["hind"])
    for name in ("wkt", "wvt", "wqt", "wot", "w1t", "w2t"):
        for l in range(L):
            if name == "wkt" and l == 0:
                continue
            nc.gpsimd.dma_start(out=w[name][l], in_=io[name][l])
    for name in ("bo", "b2", "b1_") + (("bqh",) if has_bq else ()):
        for l in range(L):
            nc.gpsimd.dma_start(out=bias[name][l], in_=io[name][l])
    nc.gpsimd.dma_start(out=penS, in_=io["pen"])
    for half in range(2):
        nc.gpsimd.dma_start(
            out=tbS[64 * half:64 * half + 64, :, :],
            in_=bass.AP(tensor=io["tgt"].tensor, offset=half * D,
                        ap=[[0, 64], [2 * D, NXT], [1, D]]))

    if DEBUG:
        nc.sync.dma_start(out=io["dbg_x0"], in_=x_all)

    def vp(i):
        # SBUF-only ops may go to Pool; PSUM-touching ops must not.
        return nc.vector if i % 2 == 0 else nc.gpsimd

    def evict2(i):
        # PSUM -> SBUF evictions: only DVE and Act can read PSUM.
        return (nc.vector.tensor_copy, nc.scalar.copy)[i % 2]

    def pe_transpose_b(dst_sbuf_slice, src, ev_fn):
        pt = psT.tile([P, P], BF16, tag="pt")
        nc.tensor.transpose(out=pt, in_=src, identity=c_idb[:])
        ev_fn(out=dst_sbuf_slice, in_=pt)

    def norm_tile(i, out_ap, in_ap, mu, rsig, nm):
        """(x - mu) * rsig, alternating DVE tensor_scalar / Act identity."""
        if i % 3 == 0:
            nc.vector.tensor_scalar(out=out_ap, in0=in_ap, scalar1=mu,
                                    scalar2=rsig, op0=OP.subtract, op1=OP.mult)
        else:
            nc.scalar.activation(out=out_ap, in_=in_ap, func=AF.Identity,
                                 scale=rsig, bias=nm)

    def combine_stats(st_all, mvb):
        """even/odd bn_stats fields -> [mu, var] without per-tile bn_aggr.

        st fields: [64, m_e, 64*v_e, 64, m_o, 64*v_o] for 128 elements.
        mu = (m_e+m_o)/2;  var = (64*v_e+64*v_o)/128 + (m_e-m_o)^2/4.
        """
        md = small.tile(list(st_all.shape[:-1]) + [1], F32, tag="md")
        nc.vector.tensor_tensor(out=md, in0=st_all[..., 1:2],
                                in1=st_all[..., 4:5], op=OP.subtract)
        nc.vector.scalar_tensor_tensor(out=md, in0=md, scalar=0.25,
                                       in1=md, op0=OP.mult, op1=OP.mult)
        nc.vector.tensor_tensor(out=mvb[..., 1:2], in0=st_all[..., 2:3],
                                in1=st_all[..., 5:6], op=OP.add)
        nc.vector.scalar_tensor_tensor(out=mvb[..., 1:2], in0=mvb[..., 1:2],
                                       scalar=1.0 / 128.0, in1=md,
                                       op0=OP.mult, op1=OP.add)
        nc.vector.tensor_tensor(out=mvb[..., 0:1], in0=st_all[..., 1:2],
                                in1=st_all[..., 4:5], op=OP.add)
        nc.vector.tensor_scalar(out=mvb[..., 0:1], in0=mvb[..., 0:1],
                                scalar1=0.5, scalar2=None, op0=OP.mult)

    # ---- stage 0: z = LN(seq) -> zT feature-major (bf16); kT for layer 0 ----
    # stats for 4 chunk-groups batched per sqrt to keep the Act table stable
    for g8 in range(NT // 2048):
        mvb = small.tile([P, 4, 4, 2], F32, tag="mvb")
        nm = small.tile([P, 4, 4, 1], F32, tag="nm")
        st_all = small.tile([P, 4, 4, 6], F32, tag="st0")
        s4s = []
        for gg in range(4):
            g4 = 4 * g8 + gg
            s4 = ld.tile([P, 512], BF16, tag="seqld", bufs=6)
            nc.sync.dma_start(out=s4, in_=bass.AP(
                tensor=io["seqb"].tensor, offset=g4 * 512 * D,
                ap=[[512, 128], [1, 512]]))
            s4s.append(s4)
            for j in range(4):
                nc.vector.bn_stats(out=st_all[:, gg, j, :],
                                   in_=s4[:, j * 128:(j + 1) * 128])
        combine_stats(st_all, mvb)
        nc.scalar.activation(out=mvb[:, :, :, 1:2], in_=mvb[:, :, :, 1:2],
                             func=AF.Sqrt, bias=c_eps[:])
        nc.vector.reciprocal(out=mvb[:, :, :, 1:2], in_=mvb[:, :, :, 1:2])
        nc.vector.scalar_tensor_tensor(
            out=nm, in0=mvb[:, :, :, 0:1], scalar=-1.0,
            in1=mvb[:, :, :, 1:2], op0=OP.mult, op1=OP.mult)
        for gg in range(4):
            g4 = 4 * g8 + gg
            s4 = s4s[gg]
            z4 = ld.tile([P, 512], BF16, tag="ztok", bufs=3)
            pt4 = psT.tile([P, 512], BF16, tag="pt")
            for j in range(4):
                norm_tile(g4 * 4 + j, z4[:, j * 128:(j + 1) * 128],
                          s4[:, j * 128:(j + 1) * 128], mvb[:, gg, j, 0:1],
                          mvb[:, gg, j, 1:2], nm[:, gg, j, :])
                nc.tensor.transpose(out=pt4[:, j * 128:(j + 1) * 128],
                                    in_=z4[:, j * 128:(j + 1) * 128],
                                    identity=c_idb[:])
            evict2(g4)(out=zT[:, g4 * 512:(g4 + 1) * 512], in_=pt4)
            ps = psA.tile([P, 512], F32, tag="mm")
            nc.tensor.matmul(out=ps, lhsT=w["wkt"][0][:],
                             rhs=zT[:, g4 * 512:(g4 + 1) * 512],
                             start=True, stop=True)
            evict2(g4 + 1)(out=kT[:, g4 * 512:(g4 + 1) * 512], in_=ps)

    if DEBUG:
        nc.sync.dma_start(out=io["dbg_zT"], in_=zT)
        nc.sync.dma_start(out=io["dbg_kT"], in_=kT)

    # ---- x layernorm -> xnT (feature-major bf16) ----
    def ln_to(dst_T):
        mvb = small.tile([P, NXT, 2], F32, tag="mvb2")
        nm = small.tile([P, NXT, 1], F32, tag="nm2")
        st_all = small.tile([P, NXT, 6], F32, tag="stl")
        for jj in range(NXT):
            nc.vector.bn_stats(out=st_all[:, jj, :], in_=x_all[:, jj, :])
        combine_stats(st_all, mvb)
        nc.scalar.activation(out=mvb[:, :, 1:2], in_=mvb[:, :, 1:2],
                             func=AF.Sqrt, bias=c_eps[:])
        nc.vector.reciprocal(out=mvb[:, :, 1:2], in_=mvb[:, :, 1:2])
        nc.vector.scalar_tensor_tensor(
            out=nm, in0=mvb[:, :, 0:1], scalar=-1.0,
            in1=mvb[:, :, 1:2], op0=OP.mult, op1=OP.mult)
        for g4 in range(NXT // 4):
            z4 = ld.tile([P, 512], BF16, tag="zx")
            pt4 = psT.tile([P, 512], BF16, tag="pt")
            for j in range(4):
                jj = 4 * g4 + j
                norm_tile(jj, z4[:, j * 128:(j + 1) * 128], x_all[:, jj, :],
                          mvb[:, jj, 0:1], mvb[:, jj, 1:2], nm[:, jj, :])
                nc.tensor.transpose(out=pt4[:, j * 128:(j + 1) * 128],
                                    in_=z4[:, j * 128:(j + 1) * 128],
                                    identity=c_idb[:])
            evict2(g4)(out=dst_T[:, g4 * 512:(g4 + 1) * 512], in_=pt4)

    for l in range(L):
        # ---- kT = Wk' @ z (feature-major); layer 0 done in stage 0 ----
        for nn in (() if l == 0 else range(NT // 512)):
            ps = psA.tile([P, 512], F32, tag="mm")
            nc.tensor.matmul(out=ps, lhsT=w["wkt"][l][:],
                             rhs=zT[:, nn * 512:(nn + 1) * 512],
                             start=True, stop=True)
            nc.scalar.copy(out=kT[:, nn * 512:(nn + 1) * 512], in_=ps)
        # ---- x LN + q projection (token-major, 2 rows x 64 groups/tile) ----
        ln_to(xnT)
        for j4 in range(NXT // 4):
            psq = psA.tile([P, 512], F32, tag="mm")
            for j in range(4):
                nc.tensor.matmul(
                    out=psq[:, j * 128:(j + 1) * 128],
                    lhsT=xnT[:, (4 * j4 + j) * 128:(4 * j4 + j + 1) * 128],
                    rhs=w["wqt"][l][:], start=True, stop=True)
            evict2(j4)(out=q_sb[:, 4 * j4:4 * j4 + 4, :], in_=psq)
        if DEBUG and l == 0:
            nc.sync.dma_start(out=io["dbg_q"], in_=q_sb)
        # ---- attention, one row pair at a time ----
        cts = []
        for rp in range(NXT):
            pc = psC.tile([P, 148], F32, tag="ctx")
            sps = pc[:, 132:148]
            for half in range(2):
                r = 2 * rp + half
                qv = psR.tile([P, 512], F32, tag="qv")
                nc.tensor.matmul(
                    out=qv[:, 0:256],
                    lhsT=q_sb[64 * half:64 * half + 64, rp, :],
                    rhs=mgS[64 * half:64 * half + 64, rp, :],
                    start=True, stop=True)
                for c in range(2):
                    cc = 2 * r + c
                    nc.tensor.matmul(
                        out=qv[:, 256 + c * 128:256 + (c + 1) * 128],
                        lhsT=zT[:, cc * 128:(cc + 1) * 128],
                        rhs=w["wvt"][l][:], start=True, stop=True)
                prod = prodp.tile([P, 2, D], BF16, tag="prod")
                nc.vector.tensor_tensor(
                    out=prod[:].rearrange("p c d -> p (c d)"),
                    in0=qv[:, 0:256],
                    in1=kT[:, 2 * r * 128:(2 * r + 2) * 128], op=OP.mult)
                for c in range(2):
                    k = 2 * half + c
                    nc.tensor.matmul(out=sps[:, k * 4:(k + 1) * 4],
                                     lhsT=prod[:, c, :], rhs=c_hind[:],
                                     start=True, stop=not has_bq)
                    if has_bq:
                        nc.tensor.matmul(
                            out=sps[:, k * 4:(k + 1) * 4],
                            lhsT=kT[:, (2 * r + c) * 128:(2 * r + c + 1) * 128],
                            rhs=bias["bqh"][l][:], start=False, stop=True)
                EV = epool.tile([P, 2, H * HD + H], BF16, tag="EV")
                nc.scalar.activation(
                    out=EV[:, :, 128:132],
                    in_=sps[:, half * 8:half * 8 + 8].rearrange(
                        "p (c h) -> p c h", c=2),
                    func=AF.Exp, scale=float(SCALE_S))
                nc.vector.tensor_tensor(
                    out=EV[:, :, 0:128].rearrange("p c (h d) -> p c h d",
                                                  h=H),
                    in0=qv[:, 256:512].rearrange("p (c h d) -> p c h d",
                                                 c=2, h=H),
                    in1=EV[:, :, 128:132].rearrange(
                        "p c (h o) -> p c h o", o=1).to_broadcast(
                            [P, 2, H, HD]),
                    op=OP.mult)
                for c in range(2):
                    nc.tensor.matmul(out=pc[64 * half:64 * half + 64, 0:132],
                                     lhsT=mtgS[:, r, c, :], rhs=EV[:, c, :],
                                     start=(c == 0), stop=(c == 1))
            rd = small.tile([P, H, 1], F32, tag="rd")
            nc.vector.tensor_scalar(
                out=rd, in0=pc[:, 128:132].rearrange("p (h o) -> p h o", o=1),
                scalar1=1e-30, scalar2=None, op0=OP.add)
            nc.vector.reciprocal(out=rd, in_=rd)
            ct = ctokp.tile([P, D], BF16, tag="ctok")
            nc.vector.scalar_tensor_tensor(
                out=ct[:].rearrange("p (h d) -> p h d", h=H),
                in0=pc[:, 0:128].rearrange("p (h d) -> p h d", h=H),
                scalar=1.0, in1=rd[:].to_broadcast([P, H, HD]),
                op0=OP.mult, op1=OP.mult)
            cts.append(ct)
            if rp % 4 == 3:
                sl = rp // 4
                cT = ev.tile([P, 512], BF16, tag="cT")
                ptc = psT.tile([P, 512], BF16, tag="pt")
                for k in range(4):
                    nc.tensor.transpose(out=ptc[:, k * 128:(k + 1) * 128],
                                        in_=cts[k][:], identity=c_idb[:])
                evict2(sl)(out=cT, in_=ptc)
                cts = []
                ps = psA.tile([P, 512], F32, tag="mm")
                nc.tensor.matmul(out=ps, lhsT=w["wot"][l][:], rhs=cT,
                                 start=True, stop=True)
                aoT = ev.tile([P, 512], BF16, tag="aoT")
                nc.scalar.activation(out=aoT, in_=ps, func=AF.Identity,
                                     bias=bias["bo"][l][:])
                pt4 = psT.tile([P, 512], BF16, tag="pt")
                for k in range(4):
                    nc.tensor.transpose(out=pt4[:, k * 128:(k + 1) * 128],
                                        in_=aoT[:, k * 128:(k + 1) * 128],
                                        identity=c_idb[:])
                nc.vector.tensor_tensor(
                    out=x_all[:, 4 * sl:4 * sl + 4, :],
                    in0=x_all[:, 4 * sl:4 * sl + 4, :],
                    in1=pt4[:].rearrange("p (j d) -> p j d", j=4), op=OP.add)

        # ---- FFN ----
        ln_to(xnT)
        for nn in range(NX // 512):
            r1 = []
            for fc in range(4):
                ps = psA.tile([P, 512], F32, tag="mm")
                nc.tensor.matmul(out=ps,
                                 lhsT=w["w1t"][l][:, fc * 128:(fc + 1) * 128],
                                 rhs=xnT[:, nn * 512:(nn + 1) * 512],
                                 start=True, stop=True)
                r1t = ev.tile([P, 512], BF16, tag="r1")
                if fc % 2 == 0:
                    nc.scalar.activation(out=r1t, in_=ps, func=AF.Relu,
                                         bias=bias["b1_"][l][:, fc:fc + 1])
                else:
                    nc.vector.tensor_scalar(out=r1t, in0=ps,
                                            scalar1=bias["b1_"][l][:, fc:fc + 1],
                                            scalar2=0.0, op0=OP.add, op1=OP.max)
                r1.append(r1t)
            ps2 = psA.tile([P, 512], F32, tag="mm")
            for fc in range(4):
                nc.tensor.matmul(out=ps2,
                                 lhsT=w["w2t"][l][:, fc * 128:(fc + 1) * 128],
                                 rhs=r1[fc], start=(fc == 0), stop=(fc == 3))
            f2T = ev.tile([P, 512], BF16, tag="aoT")
            nc.scalar.activation(out=f2T, in_=ps2, func=AF.Identity,
                                 bias=bias["b2"][l][:])
            pt4 = psT.tile([P, 512], BF16, tag="pt")
            for k in range(4):
                nc.tensor.transpose(out=pt4[:, k * 128:(k + 1) * 128],
                                    in_=f2T[:, k * 128:(k + 1) * 128],
                                    identity=c_idb[:])
            nc.vector.tensor_tensor(
                out=x_all[:, 4 * nn:4 * nn + 4, :],
                in0=x_all[:, 4 * nn:4 * nn + 4, :],
                in1=pt4[:].rearrange("p (j d) -> p j d", j=4), op=OP.add)
        if DEBUG and l == 0:
            nc.sync.dma_start(out=io["dbg_xl0"], in_=x_all)

    # ---- final stage (fp32): logits, softmax over groups, weighted sum ----
    Lpair = persist.tile([P, NXT], F32)
    for gg in range(4):
        sc = ld.tile([P, 8, D], F32, tag="fsc")
        vp(gg).tensor_tensor(out=sc, in0=x_all[:, 8 * gg:8 * gg + 8, :],
                             in1=tbS[:, 8 * gg:8 * gg + 8, :], op=OP.mult)
        nc.vector.tensor_reduce(
            out=Lpair[:, 8 * gg:8 * gg + 8].rearrange("p (j o) -> p j o", o=1),
            in_=sc, axis=mybir.AxisListType.X, op=OP.add)
    Lgr = persist.tile([P, R], F32)
    nc.vector.memset(Lgr, -1e9)
    for par in range(2):
        lg = Lgr[64 * par:64 * par + 64, :].rearrange("p (j two) -> p j two",
                                                      two=2)
        nc.vector.scalar_tensor_tensor(
            out=lg[:, :, par:par + 1],
            in0=Lpair[64 * par:64 * par + 64, :].rearrange(
                "p (j o) -> p j o", o=1),
            scalar=float(SCALE_L),
            in1=penS[64 * par:64 * par + 64, :].rearrange(
                "p (j o) -> p j o", o=1),
            op0=OP.mult, op1=OP.add)
    psL = psC.tile([R, P], F32, tag="ctx")
    nc.tensor.transpose(out=psL, in_=Lgr, identity=c_id[:])
    Erg = persist.tile([R, P], F32)
    den = small.tile([R, 1], F32, tag="den")
    nc.scalar.activation(out=Erg, in_=psL, func=AF.Exp, accum_out=den)
    nc.vector.reciprocal(out=den, in_=den)
    nc.vector.tensor_scalar(out=Erg, in0=Erg, scalar1=den, scalar2=None,
                            op0=OP.mult)
    psW = psC.tile([P, R], F32, tag="ctx")
    nc.tensor.transpose(out=psW, in_=Erg, identity=c_id[0:R, 0:R])
    wT = persist.tile([P, R], F32)
    nc.vector.tensor_copy(out=wT, in_=psW)
    for a in range(NXT // 4):
        psO = psC.tile([2, 512], F32, tag="ctx")
        for k in range(4):
            j = 4 * a + k
            nc.tensor.matmul(out=psO[:, k * 128:(k + 1) * 128],
                             lhsT=wT[:, 2 * j:2 * j + 2],
                             rhs=x_all[:, j, :], start=True, stop=True)
        o4 = ev.tile([2, 512], F32, tag="osb")
        evict2(a)(out=o4, in_=psO)
        eng = (nc.sync, nc.scalar, nc.gpsimd)[a % 3]
        eng.dma_start(
            out=bass.AP(tensor=io["out"].tensor, offset=8 * a * D,
                        ap=[[D, 2], [2 * D, 4], [1, D]]),
            in_=o4)


# ---------------------------------------------------------------------------
# host side
# ---------------------------------------------------------------------------

_NC_CACHE = {}


def _get_nc(has_bq=False):
    key = ("nc", has_bq)
    if key not in _NC_CACHE:
        nc = bacc.Bacc("TRN2", target_bir_lowering=False, debug=False,
                       enable_asserts=False)
        _build(nc, has_bq)
        nc.compile()
        _NC_CACHE[key] = nc
    return _NC_CACHE[key]


def _consts():
    ident = np.eye(128, dtype=np.float32)
    identb = np.eye(128, dtype=ml_dtypes.bfloat16)
    hind = np.zeros((128, H), np.float32)
    for h in range(H):
        hind[h * HD:(h + 1) * HD, h] = 1.0
    return dict(ident=ident, identb=identb,
                hind=np.ascontiguousarray(hind.astype(ml_dtypes.bfloat16)))


def _prep_weights(inp):
    wqkv = np.asarray(inp["wqkv"], np.float32)
    bqkv = np.asarray(inp["bqkv"], np.float32)
    wo = np.asarray(inp["wo"], np.float32)
    bo = np.asarray(inp["bo"], np.float32)
    l1g = np.asarray(inp["ln1_g"], np.float32)
    l1b = np.asarray(inp["ln1_b"], np.float32)
    l2g = np.asarray(inp["ln2_g"], np.float32)
    l2b = np.asarray(inp["ln2_b"], np.float32)
    w1 = np.asarray(inp["w1"], np.float32)
    b1 = np.asarray(inp["b1"], np.float32)
    w2 = np.asarray(inp["w2"], np.float32)
    b2 = np.asarray(inp["b2"], np.float32)
    Wq, Wk, Wv = wqkv[:, :D], wqkv[:, D:2 * D], wqkv[:, 2 * D:]
    bq_, bk_, bv_ = bqkv[:, :D], bqkv[:, D:2 * D], bqkv[:, 2 * D:]
    bf = lambda x: np.ascontiguousarray(x.astype(ml_dtypes.bfloat16))
    f32 = lambda x: np.ascontiguousarray(x.astype(np.float32))
    m = {}
    has_bq = False
    for l in range(L):
        Wqp = Wq[l] * l1g[l][None, :]
        Wkp = Wk[l] * l1g[l][None, :]
        Wvp = Wv[l] * l1g[l][None, :]
        W1p = w1[l] * l2g[l][None, :]
        bqp = Wq[l] @ l1b[l] + bq_[l]
        bvp = Wv[l] @ l1b[l] + bv_[l]
        b1p = w1[l] @ l2b[l] + b1[l]
        bop = wo[l] @ bvp + bo[l]          # v bias folded through wo
        # k bias dropped exactly: constant per (group, head) under softmax
        m[f"wkt{l}"] = bf(Wkp.T)
        m[f"wvt{l}"] = bf(Wvp.T)
        m[f"wqt{l}"] = bf(Wqp.T)
        m[f"wot{l}"] = bf(wo[l].T)
        m[f"w1t{l}"] = bf(W1p.T)
        w2tl = np.empty((128, F), np.float32)
        for fc in range(4):
            w2tl[:, fc * 128:(fc + 1) * 128] = w2[l][:, fc * 128:(fc + 1) * 128].T
        m[f"w2t{l}"] = bf(w2tl)
        m[f"bo{l}"] = f32(bop[:, None])
        m[f"b2{l}"] = f32(b2[l][:, None])
        m[f"b1_{l}"] = f32(b1p.reshape(4, 128).T)
        if np.any(bqp != 0.0):
            has_bq = True
        bqh = np.zeros((D, H), np.float32)
        for h in range(H):
            bqh[h * HD:(h + 1) * HD, h] = bqp[h * HD:(h + 1) * HD]
        m[f"bqh{l}"] = bf(bqh)
    if not has_bq:
        for l in range(L):
            del m[f"bqh{l}"]
    return m, has_bq


def _prep_row_data(catm):
    """Per-core encodings of the category/mask ints.

    catm: (R, T) int32 with -1 for masked positions.
    Returns mg (128, NXT*T) bf16, mtg (128, R*2*C) bf16, qidx (128, NXT) i32,
    pen (128, NXT) f32.
    """
    g = np.arange(C)
    match = (catm[:, None, :] == g[None, :, None])          # (R, C, T) bool
    mb = match.astype(ml_dtypes.bfloat16)
    mg = np.ascontiguousarray(
        mb.reshape(NXT, 2, C, T).transpose(1, 2, 0, 3)).reshape(128, NXT * T)
    mtg = np.ascontiguousarray(
        mb.reshape(R, C, 2, 128).transpose(3, 0, 2, 1)).reshape(128, R * 2 * C)
    pos = (np.arange(T, dtype=np.int64) + 1) * match        # (R, C, T)
    qpos = pos.max(-1)                                      # (R, C)
    qi = (np.clip(qpos - 1, 0, T - 1) +
          T * np.arange(R, dtype=np.int64)[:, None]).astype(np.int32)
    qidx = np.ascontiguousarray(
        qi.reshape(NXT, 2, C).transpose(1, 2, 0)).reshape(128, NXT)
    present = match.any(-1).astype(np.float32)              # (R, C)
    penv = (present - 1.0) * 1e9
    pen = np.ascontiguousarray(
        penv.reshape(NXT, 2, C).transpose(1, 2, 0)).reshape(128, NXT)
    return mg, mtg, qidx, pen


def kernel(**inputs):
    wm, has_bq = _prep_weights(inputs)
    nc = _get_nc(has_bq)
    cm = _consts()
    seq = np.asarray(inputs["sequence_item_emb"], np.float32)
    cat = np.asarray(inputs["sequence_cat_ids"])
    msk = np.asarray(inputs["sequence_mask"])
    tgt = np.asarray(inputs["target_item_emb"], np.float32)
    in_maps = []
    for i in range(NCORES):
        rs = slice(i * R, (i + 1) * R)
        im = dict(wm)
        im.update(cm)
        im["seq"] = np.ascontiguousarray(seq[rs].reshape(NT, D))
        im["seqb"] = np.ascontiguousarray(
            im["seq"].astype(ml_dtypes.bfloat16).reshape(32, 4, 128, D)
            .transpose(0, 2, 1, 3)).reshape(NT, D)
        catm = np.where(msk[rs], cat[rs], -1).astype(np.int32)
        mg, mtg, qidx, pen = _prep_row_data(catm)
        im["mg"], im["mtg"], im["qidx"], im["pen"] = mg, mtg, qidx, pen
        im["tgt"] = np.ascontiguousarray(tgt[rs])
        in_maps.append(im)
    res = run_bass_kernel_spmd(nc, in_maps, list(range(NCORES)))
    _NC_CACHE["last"] = res
    return np.concatenate([res.results[i]["out"] for i in range(NCORES)], axis=0)


# revision 46
# speedup vs baseline: 1.1911x; 1.0631x over previous
"""Trainium2 Bass kernel for nn_CategoryInterestAttention.

Contract: kernel(**inputs) takes FULL unsharded inputs (as produced by the
problem's setup_inputs) and returns the FULL (512, 128) float32 output.

Strategy (pure data parallel, 8 NeuronCores, 64 batch rows each):
  - Each sequence token belongs to exactly one category group, so the
    attention is a segmented softmax: per token t only the score with its
    own group matters.  Per row r:
      qtokT = q_r^T @ M          (PE; M = group-match matrix, host-encoded)
      prod  = qtokT (.) kT       (DVE/Pool elementwise)
      s     = prod^T @ head_ind  (PE; per-head segment dot products)
      e     = exp(s * scale)     (Act; written into EV[:, 128:132])
      EV    = v_psum (.) e       (DVE/Pool; v read straight from PSUM)
      ctx|den = M_t^T @ EV       (PE; 2 matmuls per row, den cols for free)
    vs the dense formulation this removes the 64x redundant exp and the
    per-(head,chunk) 33-column context matmuls.
  - The k-projection bias (incl. folded LN bias) shifts all scores of a
    group by a constant -> dropped exactly (softmax shift invariance).
    The q bias needs a per-token correction; it is only emitted when the
    host-side folded bias is nonzero (it is zero for this problem).
  - Match matrices (two layouts), query gather indices, and present-group
    penalties are host-side re-encodings of sequence_cat_ids/mask ints.
  - LayerNorm gains/biases fold into projection weights host-side; bn_stats
    batched 4 tiles wide; elementwise work split across DVE and Pool.
  - bf16 matmuls with fp32 PSUM accumulation; final target-attention fp32.
"""

import numpy as np
import ml_dtypes

import concourse.bass as bass
import concourse.bacc as bacc
import concourse.tile as tile
from concourse import mybir
from concourse.bass_utils import run_bass_kernel_spmd

F32 = mybir.dt.float32
BF16 = mybir.dt.bfloat16
I32 = mybir.dt.int32
AF = mybir.ActivationFunctionType
OP = mybir.AluOpType

B, T, D = 512, 256, 128
C, H, L, F = 64, 4, 2, 512
HD = D // H                    # 32
NCORES = 8
R = B // NCORES                # 64 rows per core
NT = R * T                     # 16384 seq tokens per core
NX = R * C                     # 4096 group tokens per core
NTC = NT // 128                # 128 seq chunks
NXT = NX // 128                # 32 x-token tiles
SCALE_S = 1.0 / np.sqrt(np.float32(HD))
SCALE_L = 1.0 / np.sqrt(np.float32(D))
EPS = 1e-5


DEBUG = False


def _build(nc, has_bq):
    seq = nc.dram_tensor("seq", [NT, D], F32, kind="ExternalInput").ap()
    seqb = nc.dram_tensor("seqb", [NT, D], BF16, kind="ExternalInput").ap()
    mg = nc.dram_tensor("mg", [128, NXT * T], BF16, kind="ExternalInput").ap()
    mtg = nc.dram_tensor("mtg", [128, R * 2 * C], BF16,
                         kind="ExternalInput").ap()
    qidx = nc.dram_tensor("qidx", [128, NXT], I32, kind="ExternalInput").ap()
    pen = nc.dram_tensor("pen", [128, NXT], F32, kind="ExternalInput").ap()
    tgt = nc.dram_tensor("tgt", [R, D], F32, kind="ExternalInput").ap()
    hind = nc.dram_tensor("hind", [128, H], BF16, kind="ExternalInput").ap()
    ident = nc.dram_tensor("ident", [128, 128], F32, kind="ExternalInput").ap()
    identb = nc.dram_tensor("identb", [128, 128], BF16,
                            kind="ExternalInput").ap()
    io = dict(seq=seq, seqb=seqb, mg=mg, mtg=mtg, qidx=qidx, pen=pen, tgt=tgt,
              hind=hind, ident=ident, identb=identb)
    io["tgtb"] = nc.dram_tensor("tgtb", [R, D], BF16,
                                kind="ExternalInput").ap()
    for name, shape, dt_ in [
        ("wkt", [D, D], BF16), ("wqt", [D, D], BF16),
        ("wot", [D, D], BF16), ("w1t", [D, F], BF16), ("w2t", [D, F], BF16),
        ("bo", [D, 1], F32), ("b2", [D, 1], F32), ("b1_", [128, 4], F32),
    ] + ([("bqh", [D, H], BF16)] if has_bq else []):
        io[name] = [nc.dram_tensor(f"{name}{l}", shape, dt_,
                                   kind="ExternalInput").ap() for l in range(L)]
    io["out"] = nc.dram_tensor("out", [R, D], F32, kind="ExternalOutput").ap()
    if DEBUG:
        for nm, shape, dt_ in [
                ("dbg_x0", [128, NXT * D], F32), ("dbg_kT", [128, NT], BF16),
                ("dbg_q", [128, NXT * D], BF16),
                ("dbg_xl0", [128, NXT * D], F32), ("dbg_zT", [128, NT], BF16)]:
            io[nm] = nc.dram_tensor(nm, shape, dt_,
                                    kind="ExternalOutput").ap()

    with tile.TileContext(nc) as tc:
        from contextlib import ExitStack
        with ExitStack() as ctx:
            _body(ctx, tc, nc, io, has_bq)
    return nc


def _body(ctx, tc, nc, io, has_bq):
    P = 128
    persist = ctx.enter_context(tc.tile_pool(name="persist", bufs=1))
    consts = ctx.enter_context(tc.tile_pool(name="consts", bufs=1))
    ld = ctx.enter_context(tc.tile_pool(name="ld", bufs=3))
    small = ctx.enter_context(tc.tile_pool(name="small", bufs=2))
    ev = ctx.enter_context(tc.tile_pool(name="ev", bufs=4))
    ctokp = ctx.enter_context(tc.tile_pool(name="ctokp", bufs=5))
    epool = ctx.enter_context(tc.tile_pool(name="epool", bufs=3))
    prodp = ctx.enter_context(tc.tile_pool(name="prodp", bufs=4))
    psA = ctx.enter_context(tc.tile_pool(name="psA", bufs=2, space="PSUM"))
    psR = ctx.enter_context(tc.tile_pool(name="psR", bufs=3, space="PSUM"))
    psC = ctx.enter_context(tc.tile_pool(name="psC", bufs=2, space="PSUM"))
    psT = ctx.enter_context(tc.tile_pool(name="psT", bufs=1, space="PSUM"))

    # ---- tiles ----
    c_id = consts.tile([P, P], F32)
    c_idb = consts.tile([P, P], BF16)
    c_hind = consts.tile([P, H], BF16)
    c_eps = consts.tile([P, 1], F32)
    nc.vector.memset(c_eps, EPS)
    w = {}
    for name in ("wkt", "wqt", "wot", "w1t", "w2t"):
        w[name] = [consts.tile(list(io[name][l].shape), BF16,
                               tag=f"{name}{l}", name=f"{name}{l}")
                   for l in range(L)]
    bias = {}
    for name in ("bo", "b2", "b1_") + (("bqh",) if has_bq else ()):
        bias[name] = [
            consts.tile(list(io[name][l].shape),
                        BF16 if name == "bqh" else F32,
                        tag=f"{name}{l}", name=f"{name}{l}")
            for l in range(L)]

    # ---- persistent data ----
    kT = persist.tile([P, NT], BF16)
    zT = persist.tile([P, NT], BF16)
    x_all = persist.tile([P, NXT, D], F32)
    xnT = persist.tile([P, NX], BF16)
    mgS = persist.tile([P, NXT, T], BF16)
    mtgS = persist.tile([P, R, 2, C], BF16)
    qidxS = persist.tile([P, NXT], I32)
    penS = persist.tile([P, NXT], F32)
    tbS = persist.tile([P, NXT, D], BF16)
    z_sb = persist.tile([P, NTC, D], BF16)

    # input DMAs: qidx first (gates the gathers), gathers head the gpsimd
    # queue, bulky weights/constants follow behind them
    nc.sync.dma_start(out=qidxS, in_=io["qidx"])
    nc.sync.dma_start(out=c_idb, in_=io["identb"])
    nc.sync.dma_start(out=w["wkt"][0], in_=io["wkt"][0])
    for j in range(NXT):
        nc.gpsimd.indirect_dma_start(
            out=x_all[:, j, :], out_offset=None, in_=io["seq"][:],
            in_offset=bass.IndirectOffsetOnAxis(ap=qidxS[:, j:j + 1], axis=0))
    nc.scalar.dma_start(out=mgS, in_=io["mg"])
    nc.scalar.dma_start(out=mtgS, in_=io["mtg"])
    nc.gpsimd.dma_start(out=c_id, in_=io["ident"])
    nc.gpsimd.dma_start(out=c_hind, in_=io["hind"])
    for name in ("wkt", "wqt", "wot", "w1t", "w2t"):
        for l in range(L):
            if name == "wkt" and l == 0:
                continue
            nc.gpsimd.dma_start(out=w[name][l], in_=io[name][l])
    for name in ("bo", "b2", "b1_") + (("bqh",) if has_bq else ()):
        for l in range(L):
            nc.gpsimd.dma_start(out=bias[name][l], in_=io[name][l])
    nc.gpsimd.dma_start(out=penS, in_=io["pen"])
    for half in range(2):
        nc.gpsimd.dma_start(
            out=tbS[64 * half:64 * half + 64, :, :],
            in_=bass.AP(tensor=io["tgtb"].tensor, offset=half * D,
                        ap=[[0, 64], [2 * D, NXT], [1, D]]))

    if DEBUG:
        nc.sync.dma_start(out=io["dbg_x0"], in_=x_all)

    def vp(i):
        # SBUF-only ops may go to Pool; PSUM-touching ops must not.
        return nc.vector if i % 2 == 0 else nc.gpsimd

    def evict2(i):
        # PSUM -> SBUF evictions: only DVE and Act can read PSUM.
        return (nc.vector.tensor_copy, nc.scalar.copy)[i % 2]

    def pe_transpose_b(dst_sbuf_slice, src, ev_fn):
        pt = psT.tile([P, P], BF16, tag="pt")
        nc.tensor.transpose(out=pt, in_=src, identity=c_idb[:])
        ev_fn(out=dst_sbuf_slice, in_=pt)

    def norm_tile(i, out_ap, in_ap, mu, rsig, nm):
        """(x - mu) * rsig, alternating DVE tensor_scalar / Act identity."""
        if i % 3 == 0:
            nc.vector.tensor_scalar(out=out_ap, in0=in_ap, scalar1=mu,
                                    scalar2=rsig, op0=OP.subtract, op1=OP.mult)
        else:
            nc.scalar.activation(out=out_ap, in_=in_ap, func=AF.Identity,
                                 scale=rsig, bias=nm)

    def combine_stats(st_all, mvb):
        """even/odd bn_stats fields -> [mu, var] without per-tile bn_aggr.

        st fields: [64, m_e, 64*v_e, 64, m_o, 64*v_o] for 128 elements.
        mu = (m_e+m_o)/2;  var = (64*v_e+64*v_o)/128 + (m_e-m_o)^2/4.
        """
        md = small.tile(list(st_all.shape[:-1]) + [1], F32, tag="md")
        nc.vector.tensor_tensor(out=md, in0=st_all[..., 1:2],
                                in1=st_all[..., 4:5], op=OP.subtract)
        nc.vector.scalar_tensor_tensor(out=md, in0=md, scalar=0.25,
                                       in1=md, op0=OP.mult, op1=OP.mult)
        nc.vector.tensor_tensor(out=mvb[..., 1:2], in0=st_all[..., 2:3],
                                in1=st_all[..., 5:6], op=OP.add)
        nc.vector.scalar_tensor_tensor(out=mvb[..., 1:2], in0=mvb[..., 1:2],
                                       scalar=1.0 / 128.0, in1=md,
                                       op0=OP.mult, op1=OP.add)
        nc.vector.tensor_tensor(out=mvb[..., 0:1], in0=st_all[..., 1:2],
                                in1=st_all[..., 4:5], op=OP.add)
        nc.vector.tensor_scalar(out=mvb[..., 0:1], in0=mvb[..., 0:1],
                                scalar1=0.5, scalar2=None, op0=OP.mult)

    # ---- stage 0: z = LN(seq) -> zT feature-major (bf16); kT for layer 0 ----
    # stats for 4 chunk-groups batched per sqrt to keep the Act table stable
    for g8 in range(NT // 2048):
        mvb = small.tile([P, 4, 4, 2], F32, tag="mvb")
        nm = small.tile([P, 4, 4, 1], F32, tag="nm")
        st_all = small.tile([P, 4, 4, 6], F32, tag="st0")
        s4s = []
        for gg in range(4):
            g4 = 4 * g8 + gg
            s4 = ld.tile([P, 512], BF16, tag="seqld", bufs=5)
            nc.sync.dma_start(out=s4, in_=bass.AP(
                tensor=io["seqb"].tensor, offset=g4 * 512 * D,
                ap=[[512, 128], [1, 512]]))
            s4s.append(s4)
            for j in range(4):
                nc.vector.bn_stats(out=st_all[:, gg, j, :],
                                   in_=s4[:, j * 128:(j + 1) * 128])
        combine_stats(st_all, mvb)
        nc.scalar.activation(out=mvb[:, :, :, 1:2], in_=mvb[:, :, :, 1:2],
                             func=AF.Sqrt, bias=c_eps[:])
        nc.vector.reciprocal(out=mvb[:, :, :, 1:2], in_=mvb[:, :, :, 1:2])
        nc.vector.scalar_tensor_tensor(
            out=nm, in0=mvb[:, :, :, 0:1], scalar=-1.0,
            in1=mvb[:, :, :, 1:2], op0=OP.mult, op1=OP.mult)
        for gg in range(4):
            g4 = 4 * g8 + gg
            s4 = s4s[gg]
            pt4 = psT.tile([P, 512], BF16, tag="pt")
            for j in range(4):
                cc = 4 * g4 + j
                norm_tile(g4 * 4 + j, z_sb[:, cc, :],
                          s4[:, j * 128:(j + 1) * 128], mvb[:, gg, j, 0:1],
                          mvb[:, gg, j, 1:2], nm[:, gg, j, :])
                nc.tensor.transpose(out=pt4[:, j * 128:(j + 1) * 128],
                                    in_=z_sb[:, cc, :],
                                    identity=c_idb[:])
            evict2(g4)(out=zT[:, g4 * 512:(g4 + 1) * 512], in_=pt4)
            ps = psA.tile([P, 512], F32, tag="mm")
            nc.tensor.matmul(out=ps, lhsT=w["wkt"][0][:],
                             rhs=zT[:, g4 * 512:(g4 + 1) * 512],
                             start=True, stop=True)
            evict2(g4 + 1)(out=kT[:, g4 * 512:(g4 + 1) * 512], in_=ps)

    if DEBUG:
        nc.sync.dma_start(out=io["dbg_zT"], in_=zT)
        nc.sync.dma_start(out=io["dbg_kT"], in_=kT)

    # ---- x layernorm -> xnT (feature-major bf16) ----
    def ln_to(dst_T):
        mvb = small.tile([P, NXT, 2], F32, tag="mvb2")
        nm = small.tile([P, NXT, 1], F32, tag="nm2")
        st_all = small.tile([P, NXT, 6], F32, tag="stl")
        for jj in range(NXT):
            nc.vector.bn_stats(out=st_all[:, jj, :], in_=x_all[:, jj, :])
        combine_stats(st_all, mvb)
        nc.scalar.activation(out=mvb[:, :, 1:2], in_=mvb[:, :, 1:2],
                             func=AF.Sqrt, bias=c_eps[:])
        nc.vector.reciprocal(out=mvb[:, :, 1:2], in_=mvb[:, :, 1:2])
        nc.vector.scalar_tensor_tensor(
            out=nm, in0=mvb[:, :, 0:1], scalar=-1.0,
            in1=mvb[:, :, 1:2], op0=OP.mult, op1=OP.mult)
        for g4 in range(NXT // 4):
            z4 = ld.tile([P, 512], BF16, tag="zx", bufs=2)
            pt4 = psT.tile([P, 512], BF16, tag="pt")
            for j in range(4):
                jj = 4 * g4 + j
                norm_tile(jj, z4[:, j * 128:(j + 1) * 128], x_all[:, jj, :],
                          mvb[:, jj, 0:1], mvb[:, jj, 1:2], nm[:, jj, :])
                nc.tensor.transpose(out=pt4[:, j * 128:(j + 1) * 128],
                                    in_=z4[:, j * 128:(j + 1) * 128],
                                    identity=c_idb[:])
            evict2(g4)(out=dst_T[:, g4 * 512:(g4 + 1) * 512], in_=pt4)

    for l in range(L):
        # ---- kT = Wk' @ z (feature-major); layer 0 done in stage 0 ----
        for nn in (() if l == 0 else range(NT // 512)):
            ps = psA.tile([P, 512], F32, tag="mm")
            nc.tensor.matmul(out=ps, lhsT=w["wkt"][l][:],
                             rhs=zT[:, nn * 512:(nn + 1) * 512],
                             start=True, stop=True)
            nc.scalar.copy(out=kT[:, nn * 512:(nn + 1) * 512], in_=ps)
        # ---- x LN + q projection (token-major, 2 rows x 64 groups/tile) ----
        ln_to(xnT)
        for j4 in range(NXT // 4):
            psq = psA.tile([P, 512], F32, tag="mm")
            for j in range(4):
                nc.tensor.matmul(
                    out=psq[:, j * 128:(j + 1) * 128],
                    lhsT=xnT[:, (4 * j4 + j) * 128:(4 * j4 + j + 1) * 128],
                    rhs=w["wqt"][l][:], start=True, stop=True)
            evict2(j4)(out=xnT[:, j4 * 512:(j4 + 1) * 512], in_=psq)
        if DEBUG and l == 0:
            nc.sync.dma_start(out=io["dbg_q"], in_=q_sb)
        # ---- attention, one row pair at a time ----
        cts = []
        for rp in range(NXT):
            pc = psC.tile([P, 148], F32, tag="ctx")
            sps = pc[:, 132:148]
            for half in range(2):
                r = 2 * rp + half
                qv = psR.tile([P, 256], F32, tag="qv")
                nc.tensor.matmul(
                    out=qv,
                    lhsT=xnT[64 * half:64 * half + 64,
                             rp * 128:(rp + 1) * 128],
                    rhs=mgS[64 * half:64 * half + 64, rp, :],
                    start=True, stop=True)
                prod = prodp.tile([P, 2, D], BF16, tag="prod")
                nc.vector.tensor_tensor(
                    out=prod[:].rearrange("p c d -> p (c d)"),
                    in0=qv[:],
                    in1=kT[:, 2 * r * 128:(2 * r + 2) * 128], op=OP.mult)
                for c in range(2):
                    k = 2 * half + c
                    nc.tensor.matmul(out=sps[:, k * 4:(k + 1) * 4],
                                     lhsT=prod[:, c, :], rhs=c_hind[:],
                                     start=True, stop=not has_bq)
                    if has_bq:
                        nc.tensor.matmul(
                            out=sps[:, k * 4:(k + 1) * 4],
                            lhsT=kT[:, (2 * r + c) * 128:(2 * r + c + 1) * 128],
                            rhs=bias["bqh"][l][:], start=False, stop=True)
                EV = epool.tile([P, 2, H * HD + H], BF16, tag="EV")
                nc.scalar.activation(
                    out=EV[:, :, 128:132],
                    in_=sps[:, half * 8:half * 8 + 8].rearrange(
                        "p (c h) -> p c h", c=2),
                    func=AF.Exp, scale=float(SCALE_S))
                nc.vector.tensor_tensor(
                    out=EV[:, :, 0:128].rearrange("p c (h d) -> p c h d",
                                                  h=H),
                    in0=z_sb[:, 2 * r:2 * r + 2, :].rearrange(
                        "p c (h d) -> p c h d", h=H),
                    in1=EV[:, :, 128:132].rearrange(
                        "p c (h o) -> p c h o", o=1).to_broadcast(
                            [P, 2, H, HD]),
                    op=OP.mult)
                for c in range(2):
                    nc.tensor.matmul(out=pc[64 * half:64 * half + 64, 0:132],
                                     lhsT=mtgS[:, r, c, :], rhs=EV[:, c, :],
                                     start=(c == 0), stop=(c == 1))
            rd = small.tile([P, H, 1], F32, tag="rd")
            nc.vector.tensor_scalar(
                out=rd, in0=pc[:, 128:132].rearrange("p (h o) -> p h o", o=1),
                scalar1=1e-30, scalar2=None, op0=OP.add)
            nc.vector.reciprocal(out=rd, in_=rd)
            ct = ctokp.tile([P, D], BF16, tag="ctok")
            nc.vector.scalar_tensor_tensor(
                out=ct[:].rearrange("p (h d) -> p h d", h=H),
                in0=pc[:, 0:128].rearrange("p (h d) -> p h d", h=H),
                scalar=1.0, in1=rd[:].to_broadcast([P, H, HD]),
                op0=OP.mult, op1=OP.mult)
            cts.append(ct)
            if rp % 4 == 3:
                sl = rp // 4
                cT = ev.tile([P, 512], BF16, tag="cT")
                ptc = psT.tile([P, 512], BF16, tag="pt")
                for k in range(4):
                    nc.tensor.transpose(out=ptc[:, k * 128:(k + 1) * 128],
                                        in_=cts[k][:], identity=c_idb[:])
                evict2(sl)(out=cT, in_=ptc)
                cts = []
                ps = psA.tile([P, 512], F32, tag="mm")
                nc.tensor.matmul(out=ps, lhsT=w["wot"][l][:], rhs=cT,
                                 start=True, stop=True)
                aoT = ev.tile([P, 512], BF16, tag="aoT")
                nc.scalar.activation(out=aoT, in_=ps, func=AF.Identity,
                                     bias=bias["bo"][l][:])
                pt4 = psT.tile([P, 512], BF16, tag="pt")
                for k in range(4):
                    nc.tensor.transpose(out=pt4[:, k * 128:(k + 1) * 128],
                                        in_=aoT[:, k * 128:(k + 1) * 128],
                                        identity=c_idb[:])
                nc.vector.tensor_tensor(
                    out=x_all[:, 4 * sl:4 * sl + 4, :],
                    in0=x_all[:, 4 * sl:4 * sl + 4, :],
                    in1=pt4[:].rearrange("p (j d) -> p j d", j=4), op=OP.add)

        # ---- FFN ----
        ln_to(xnT)
        for nn in range(NX // 512):
            r1 = []
            for fc in range(4):
                ps = psA.tile([P, 512], F32, tag="mm")
                nc.tensor.matmul(out=ps,
                                 lhsT=w["w1t"][l][:, fc * 128:(fc + 1) * 128],
                                 rhs=xnT[:, nn * 512:(nn + 1) * 512],
                                 start=True, stop=True)
                r1t = ev.tile([P, 512], BF16, tag="r1")
                if fc % 2 == 0:
                    nc.scalar.activation(out=r1t, in_=ps, func=AF.Relu,
                                         bias=bias["b1_"][l][:, fc:fc + 1])
                else:
                    nc.vector.tensor_scalar(out=r1t, in0=ps,
                                            scalar1=bias["b1_"][l][:, fc:fc + 1],
                                            scalar2=0.0, op0=OP.add, op1=OP.max)
                r1.append(r1t)
            ps2 = psA.tile([P, 512], F32, tag="mm")
            for fc in range(4):
                nc.tensor.matmul(out=ps2,
                                 lhsT=w["w2t"][l][:, fc * 128:(fc + 1) * 128],
                                 rhs=r1[fc], start=(fc == 0), stop=(fc == 3))
            f2T = ev.tile([P, 512], BF16, tag="aoT")
            nc.scalar.activation(out=f2T, in_=ps2, func=AF.Identity,
                                 bias=bias["b2"][l][:])
            pt4 = psT.tile([P, 512], BF16, tag="pt")
            for k in range(4):
                nc.tensor.transpose(out=pt4[:, k * 128:(k + 1) * 128],
                                    in_=f2T[:, k * 128:(k + 1) * 128],
                                    identity=c_idb[:])
            nc.vector.tensor_tensor(
                out=x_all[:, 4 * nn:4 * nn + 4, :],
                in0=x_all[:, 4 * nn:4 * nn + 4, :],
                in1=pt4[:].rearrange("p (j d) -> p j d", j=4), op=OP.add)
        if DEBUG and l == 0:
            nc.sync.dma_start(out=io["dbg_xl0"], in_=x_all)

    # ---- final stage (fp32): logits, softmax over groups, weighted sum ----
    Lpair = persist.tile([P, NXT], F32)
    for gg in range(8):
        sc = ld.tile([P, 4, D], F32, tag="fsc", bufs=2)
        nc.vector.tensor_tensor(out=sc, in0=x_all[:, 4 * gg:4 * gg + 4, :],
                                in1=tbS[:, 4 * gg:4 * gg + 4, :], op=OP.mult)
        nc.vector.tensor_reduce(
            out=Lpair[:, 4 * gg:4 * gg + 4].rearrange("p (j o) -> p j o", o=1),
            in_=sc, axis=mybir.AxisListType.X, op=OP.add)
    Lgr = persist.tile([P, R], F32)
    nc.vector.memset(Lgr, -1e9)
    for par in range(2):
        lg = Lgr[64 * par:64 * par + 64, :].rearrange("p (j two) -> p j two",
                                                      two=2)
        nc.vector.scalar_tensor_tensor(
            out=lg[:, :, par:par + 1],
            in0=Lpair[64 * par:64 * par + 64, :].rearrange(
                "p (j o) -> p j o", o=1),
            scalar=float(SCALE_L),
            in1=penS[64 * par:64 * par + 64, :].rearrange(
                "p (j o) -> p j o", o=1),
            op0=OP.mult, op1=OP.add)
    psL = psC.tile([R, P], F32, tag="ctx")
    nc.tensor.transpose(out=psL, in_=Lgr, identity=c_id[:])
    Erg = persist.tile([R, P], F32)
    den = small.tile([R, 1], F32, tag="den")
    nc.scalar.activation(out=Erg, in_=psL, func=AF.Exp, accum_out=den)
    nc.vector.reciprocal(out=den, in_=den)
    nc.vector.tensor_scalar(out=Erg, in0=Erg, scalar1=den, scalar2=None,
                            op0=OP.mult)
    psW = psC.tile([P, R], F32, tag="ctx")
    nc.tensor.transpose(out=psW, in_=Erg, identity=c_id[0:R, 0:R])
    wT = persist.tile([P, R], F32)
    nc.vector.tensor_copy(out=wT, in_=psW)
    for a in range(NXT // 4):
        psO = psC.tile([2, 512], F32, tag="ctx")
        for k in range(4):
            j = 4 * a + k
            nc.tensor.matmul(out=psO[:, k * 128:(k + 1) * 128],
                             lhsT=wT[:, 2 * j:2 * j + 2],
                             rhs=x_all[:, j, :], start=True, stop=True)
        o4 = ev.tile([2, 512], F32, tag="osb")
        evict2(a)(out=o4, in_=psO)
        eng = (nc.sync, nc.scalar, nc.gpsimd)[a % 3]
        eng.dma_start(
            out=bass.AP(tensor=io["out"].tensor, offset=8 * a * D,
                        ap=[[D, 2], [2 * D, 4], [1, D]]),
            in_=o4)


# ---------------------------------------------------------------------------
# host side
# ---------------------------------------------------------------------------

_NC_CACHE = {}


def _get_nc(has_bq=False):
    key = ("nc", has_bq)
    if key not in _NC_CACHE:
        nc = bacc.Bacc("TRN2", target_bir_lowering=False, debug=False,
                       enable_asserts=False)
        _build(nc, has_bq)
        nc.compile()
        _NC_CACHE[key] = nc
    return _NC_CACHE[key]


def _consts():
    ident = np.eye(128, dtype=np.float32)
    identb = np.eye(128, dtype=ml_dtypes.bfloat16)
    hind = np.zeros((128, H), np.float32)
    for h in range(H):
        hind[h * HD:(h + 1) * HD, h] = 1.0
    return dict(ident=ident, identb=identb,
                hind=np.ascontiguousarray(hind.astype(ml_dtypes.bfloat16)))


def _prep_weights(inp):
    wqkv = np.asarray(inp["wqkv"], np.float32)
    bqkv = np.asarray(inp["bqkv"], np.float32)
    wo = np.asarray(inp["wo"], np.float32)
    bo = np.asarray(inp["bo"], np.float32)
    l1g = np.asarray(inp["ln1_g"], np.float32)
    l1b = np.asarray(inp["ln1_b"], np.float32)
    l2g = np.asarray(inp["ln2_g"], np.float32)
    l2b = np.asarray(inp["ln2_b"], np.float32)
    w1 = np.asarray(inp["w1"], np.float32)
    b1 = np.asarray(inp["b1"], np.float32)
    w2 = np.asarray(inp["w2"], np.float32)
    b2 = np.asarray(inp["b2"], np.float32)
    Wq, Wk, Wv = wqkv[:, :D], wqkv[:, D:2 * D], wqkv[:, 2 * D:]
    bq_, bk_, bv_ = bqkv[:, :D], bqkv[:, D:2 * D], bqkv[:, 2 * D:]
    bf = lambda x: np.ascontiguousarray(x.astype(ml_dtypes.bfloat16))
    f32 = lambda x: np.ascontiguousarray(x.astype(np.float32))
    m = {}
    has_bq = False
    for l in range(L):
        Wqp = Wq[l] * l1g[l][None, :]
        Wkp = Wk[l] * l1g[l][None, :]
        Wvp = Wv[l] * l1g[l][None, :]
        W1p = w1[l] * l2g[l][None, :]
        bqp = Wq[l] @ l1b[l] + bq_[l]
        bvp = Wv[l] @ l1b[l] + bv_[l]
        b1p = w1[l] @ l2b[l] + b1[l]
        bop = wo[l] @ bvp + bo[l]          # v bias folded through wo
        # k bias dropped exactly: constant per (group, head) under softmax
        m[f"wkt{l}"] = bf(Wkp.T)
        m[f"wqt{l}"] = bf(Wqp.T)
        m[f"wot{l}"] = bf((wo[l] @ Wvp).T)
        m[f"w1t{l}"] = bf(W1p.T)
        w2tl = np.empty((128, F), np.float32)
        for fc in range(4):
            w2tl[:, fc * 128:(fc + 1) * 128] = w2[l][:, fc * 128:(fc + 1) * 128].T
        m[f"w2t{l}"] = bf(w2tl)
        m[f"bo{l}"] = f32(bop[:, None])
        m[f"b2{l}"] = f32(b2[l][:, None])
        m[f"b1_{l}"] = f32(b1p.reshape(4, 128).T)
        if np.any(bqp != 0.0):
            has_bq = True
        bqh = np.zeros((D, H), np.float32)
        for h in range(H):
            bqh[h * HD:(h + 1) * HD, h] = bqp[h * HD:(h + 1) * HD]
        m[f"bqh{l}"] = bf(bqh)
    if not has_bq:
        for l in range(L):
            del m[f"bqh{l}"]
    return m, has_bq


def _prep_row_data(catm):
    """Per-core encodings of the category/mask ints.

    catm: (R, T) int32 with -1 for masked positions.
    Returns mg (128, NXT*T) bf16, mtg (128, R*2*C) bf16, qidx (128, NXT) i32,
    pen (128, NXT) f32.
    """
    g = np.arange(C)
    match = (catm[:, None, :] == g[None, :, None])          # (R, C, T) bool
    mb = match.astype(ml_dtypes.bfloat16)
    mg = np.ascontiguousarray(
        mb.reshape(NXT, 2, C, T).transpose(1, 2, 0, 3)).reshape(128, NXT * T)
    mtg = np.ascontiguousarray(
        mb.reshape(R, C, 2, 128).transpose(3, 0, 2, 1)).reshape(128, R * 2 * C)
    pos = (np.arange(T, dtype=np.int64) + 1) * match        # (R, C, T)
    qpos = pos.max(-1)                                      # (R, C)
    qi = (np.clip(qpos - 1, 0, T - 1) +
          T * np.arange(R, dtype=np.int64)[:, None]).astype(np.int32)
    qidx = np.ascontiguousarray(
        qi.reshape(NXT, 2, C).transpose(1, 2, 0)).reshape(128, NXT)
    present = match.any(-1).astype(np.float32)              # (R, C)
    penv = (present - 1.0) * 1e9
    pen = np.ascontiguousarray(
        penv.reshape(NXT, 2, C).transpose(1, 2, 0)).reshape(128, NXT)
    return mg, mtg, qidx, pen


def kernel(**inputs):
    wm, has_bq = _prep_weights(inputs)
    nc = _get_nc(has_bq)
    cm = _consts()
    seq = np.asarray(inputs["sequence_item_emb"], np.float32)
    cat = np.asarray(inputs["sequence_cat_ids"])
    msk = np.asarray(inputs["sequence_mask"])
    tgt = np.asarray(inputs["target_item_emb"], np.float32)
    in_maps = []
    for i in range(NCORES):
        rs = slice(i * R, (i + 1) * R)
        im = dict(wm)
        im.update(cm)
        im["seq"] = np.ascontiguousarray(seq[rs].reshape(NT, D))
        im["seqb"] = np.ascontiguousarray(
            im["seq"].astype(ml_dtypes.bfloat16).reshape(32, 4, 128, D)
            .transpose(0, 2, 1, 3)).reshape(NT, D)
        catm = np.where(msk[rs], cat[rs], -1).astype(np.int32)
        mg, mtg, qidx, pen = _prep_row_data(catm)
        im["mg"], im["mtg"], im["qidx"], im["pen"] = mg, mtg, qidx, pen
        im["tgt"] = np.ascontiguousarray(tgt[rs])
        im["tgtb"] = np.ascontiguousarray(tgt[rs].astype(ml_dtypes.bfloat16))
        in_maps.append(im)
    res = run_bass_kernel_spmd(nc, in_maps, list(range(NCORES)))
    _NC_CACHE["last"] = res
    return np.concatenate([res.results[i]["out"] for i in range(NCORES)], axis=0)
